# revision 1
# baseline (speedup 1.0000x reference)
"""Raw-bass embedding lookup for TRN2: out[i] = feature_array[int(x[i,0])].

Data-parallel over N across 8 NeuronCores; the [512, 64] table is replicated.
Host side converts the float case-IDs to int32 and pads each 25000-row shard
to 25088 = 128*196, laid out so SBUF partition p owns output rows
p*196 .. p*196+195.

A HW probe showed indirect InstDMACopy honors only one index per partition
(128 descriptors) per instruction, so each s-column is its own gather
(table rows land per-partition-contiguous in SBUF). Gathers pipeline through
a rotating 7-semaphore window (7 in flight stays under the 1024-descriptor
SWDGE ring); writebacks batch 28 s-columns into one contiguous-per-partition
HWDGE DMA (7KB/partition) once their gathers complete, overlapping later
gathers. Every instruction carries at most one semaphore wait (this walrus
build rejects more).
"""

import numpy as np

N = 200_000
C = 512
D = 64
NCORES = 8
NS = N // NCORES
P = 128
S = 196
SP = P * S
NSEM = 7
WB = 28  # s-columns per writeback (196 = 7*28); NSEM divides WB
NWB = S // WB

_RUN_OPTS: dict = {}
_LAST_RESULT = None
_LAST_IN_MAPS = None
_NC_CACHE = None


def _build():
    global _NC_CACHE
    if _NC_CACHE is not None:
        return _NC_CACHE
    import concourse.bass as bass
    import concourse.mybir as mybir
    from contextlib import ExitStack

    nc = bass.Bass()
    x = nc.dram_tensor("x", [P, S], mybir.dt.int32, kind="ExternalInput")
    feat = nc.dram_tensor("feature", [C, D], mybir.dt.float32, kind="ExternalInput")
    out = nc.dram_tensor("out", [SP, D], mybir.dt.float32, kind="ExternalOutput")
    out_v = out[:].rearrange("(p s) d -> p (s d)", p=P)

    with (
        ExitStack() as stack,
        nc.sbuf_tensor("xi", [P, S], mybir.dt.int32) as xi,
        nc.sbuf_tensor("g", [P, S * D], mybir.dt.float32) as g,
        nc.semaphore("s_load") as s_load,
        nc.Block() as block,
    ):
        s_gath = [stack.enter_context(nc.semaphore(f"s_g{k}")) for k in range(NSEM)]
        s_out = [stack.enter_context(nc.semaphore(f"s_o{k}")) for k in range(NWB)]

        @block.sync
        def _(sync):
            sync.dma_start(out=xi[:], in_=x[:]).then_inc(s_load, 16)
            for w in range(NWB):
                # window w covers s < 28*(w+1); each of the 7 sems has had
                # exactly 4*(w+1) increments of 16 by then
                for k in range(NSEM):
                    sync.wait_ge(s_gath[k], 16 * (WB // NSEM) * (w + 1))
                sync.dma_start(
                    out=out_v[:, w * WB * D : (w + 1) * WB * D],
                    in_=g[:, w * WB * D : (w + 1) * WB * D],
                ).then_inc(s_out[w], 16)
            for w in range(NWB):
                sync.wait_ge(s_out[w], 16)

        @block.gpsimd
        def _(gpsimd):
            gpsimd.wait_ge(s_load, 16)
            for s in range(S):
                k, r = s % NSEM, s // NSEM
                if r > 0:
                    gpsimd.wait_ge(s_gath[k], 16 * r)
                gpsimd.indirect_dma_start(
                    out=g[:, s * D : (s + 1) * D],
                    out_offset=None,
                    in_=feat[:],
                    in_offset=bass.IndirectOffsetOnAxis(
                        ap=xi[:, s : s + 1], axis=0
                    ),
                ).then_inc(s_gath[k], 16)

    _NC_CACHE = nc
    return nc


def kernel(x, feature_array):
    global _LAST_RESULT, _LAST_IN_MAPS
    from concourse.bass_utils import run_bass_kernel_spmd

    nc = _build()
    xs = np.asarray(x).reshape(NCORES, NS).astype(np.int32)
    feat = np.ascontiguousarray(np.asarray(feature_array, dtype=np.float32))
    in_maps = []
    for i in range(NCORES):
        xp = np.zeros((P, S), dtype=np.int32)
        xp.reshape(-1)[:NS] = xs[i]
        in_maps.append({"x": xp, "feature": feat})
    _LAST_IN_MAPS = in_maps
    res = run_bass_kernel_spmd(nc, in_maps, core_ids=list(range(NCORES)), **_RUN_OPTS)
    _LAST_RESULT = res
    return np.concatenate([r["out"][:NS] for r in res.results], axis=0)



# revision 2
# speedup vs baseline: 1.2191x; 1.2191x over previous
"""Raw-bass embedding lookup for TRN2: out[i] = feature_array[int(x[i,0])].

Data-parallel over N across 8 NeuronCores; the [512, 64] table is replicated.
Host side converts the float case-IDs to int16 indices and pads each
25000-row shard to 25088 = 128*196, laid out so SBUF partition p owns output
rows p*196 .. p*196+195.

The gather uses the GPSIMD `dma_gather` extended instruction (mlp ucode
library) instead of per-s-column indirect InstDMACopy: one instruction
gathers 896 table rows (7 s-columns * 128 partitions) with descriptor
generation at ~0.34 ns/desc on the Q7 CounterMachine, vs ~1 us fixed cost
per 128-row indirect DMA. 28 chunks cover the shard; each stays under the
1024-descriptor SWDGE ring (larger chunks crash the exec unit; the ring
size is fixed in ucode regardless of dynamic_dma_scratch_size). Index
layout per dma_gather semantics: logical gather position i (out slot
p=i%128, s-col i//128) reads idxs_sb[i%16, i//16], replicated across the
8 groups of 16 partitions (one per Q7 core). Writebacks batch 28 s-columns
(4 chunks) into one contiguous-per-partition HWDGE DMA, overlapping later
gathers. Every instruction carries at most one semaphore wait (this walrus
build rejects more).
"""

import numpy as np

N = 200_000
C = 512
D = 64
NCORES = 8
NS = N // NCORES
P = 128
S = 196
SP = P * S
GW = 7  # s-columns per dma_gather chunk (896 descs < 1024 ring)
NCH = S // GW  # 28 gather chunks
CHUNK = P * GW  # 896 indices per chunk
WB = 28  # s-columns per writeback (= 4 gather chunks)
NWB = S // WB

_RUN_OPTS: dict = {}
_LAST_RESULT = None
_LAST_IN_MAPS = None
_NC_CACHE = None


def _build():
    global _NC_CACHE
    if _NC_CACHE is not None:
        return _NC_CACHE
    import concourse.bass as bass
    import concourse.mybir as mybir
    from concourse import library_config
    from concourse.library_overlay import lower_extended_insts

    nc = bass.Bass()
    idx = nc.dram_tensor("idx", [P, SP // 16], mybir.dt.int16, kind="ExternalInput")
    feat = nc.dram_tensor("feature", [C, D], mybir.dt.float32, kind="ExternalInput")
    out = nc.dram_tensor("out", [SP, D], mybir.dt.float32, kind="ExternalOutput")
    out_v = out[:].rearrange("(p s) d -> p (s d)", p=P)

    with (
        nc.sbuf_tensor("xi", [P, SP // 16], mybir.dt.int16) as xi,
        nc.sbuf_tensor("g", [P, S * D], mybir.dt.float32) as g,
        nc.semaphore("s_load") as s_load,
        nc.semaphore("s_gath") as s_gath,
        nc.semaphore("s_out") as s_out,
        nc.Block() as block,
    ):
        @block.sync
        def _(sync):
            sync.dma_start(out=xi[:], in_=idx[:]).then_inc(s_load, 16)
            for w in range(NWB):
                # chunks complete in FIFO ring order; writeback w needs the
                # first 4*(w+1) chunks done
                sync.wait_ge(s_gath, 16 * (WB // GW) * (w + 1))
                sync.dma_start(
                    out=out_v[:, w * WB * D : (w + 1) * WB * D],
                    in_=g[:, w * WB * D : (w + 1) * WB * D],
                ).then_inc(s_out, 16)
            sync.wait_ge(s_out, 16 * NWB)

        @block.gpsimd
        def _(gpsimd):
            gpsimd.load_library(library_config.mlp)
            gpsimd.wait_ge(s_load, 16)
            for j in range(NCH):
                gpsimd.dma_gather(
                    g[:, j * GW * D : (j + 1) * GW * D].rearrange(
                        "p (s d) -> p s d", d=D
                    ),
                    feat[:],
                    xi[:, j * (CHUNK // 16) : (j + 1) * (CHUNK // 16)],
                    CHUNK,
                    CHUNK,
                    D,
                ).then_inc(s_gath, 16)

    lower_extended_insts(nc)
    _NC_CACHE = nc
    return nc


def kernel(x, feature_array):
    global _LAST_RESULT, _LAST_IN_MAPS
    from concourse.bass_utils import run_bass_kernel_spmd

    nc = _build()
    xs = np.asarray(x).reshape(NCORES, NS).astype(np.int16)
    feat = np.ascontiguousarray(np.asarray(feature_array, dtype=np.float32))
    in_maps = []
    for i in range(NCORES):
        xp = np.zeros((P, S), dtype=np.int16)
        xp.reshape(-1)[:NS] = xs[i]
        # logical gather position i = s*128 + p -> out slot [p, s] -> row
        # p*196+s, so idx_flat = xp.T.ravel(); wrap by 16 and replicate
        # across the 8 Q7-core partition groups.
        idx_flat = np.ascontiguousarray(xp.T).ravel()
        base = idx_flat.reshape(SP // 16, 16).T
        idx_sb = np.tile(base, (8, 1))
        in_maps.append({"idx": np.ascontiguousarray(idx_sb), "feature": feat})
    _LAST_IN_MAPS = in_maps
    res = run_bass_kernel_spmd(nc, in_maps, core_ids=list(range(NCORES)), **_RUN_OPTS)
    _LAST_RESULT = res
    return np.concatenate([r["out"][:NS] for r in res.results], axis=0)


# revision 3
# speedup vs baseline: 1.2390x; 1.0163x over previous
"""Raw-bass embedding lookup for TRN2: out[i] = feature_array[int(x[i,0])].

Data-parallel over N across 8 NeuronCores; the [512, 64] table is replicated.
Host side converts the float case-IDs to int16 indices and pads each
25000-row shard to 25088 = 128*196, laid out so SBUF partition p owns output
rows p*196 .. p*196+195.

The gather uses the GPSIMD `dma_gather` extended instruction (mlp ucode
library) instead of per-s-column indirect InstDMACopy: one instruction
gathers 896 table rows (7 s-columns * 128 partitions) with descriptor
generation at ~0.34 ns/desc on the Q7 CounterMachine, vs ~1 us fixed cost
per 128-row indirect DMA. 28 chunks cover the shard; each stays under the
1024-descriptor SWDGE ring (larger chunks crash the exec unit; the ring
size is fixed in ucode regardless of dynamic_dma_scratch_size). Index
layout per dma_gather semantics: logical gather position i (out slot
p=i%128, s-col i//128) reads idxs_sb[i%16, i//16], replicated across the
8 groups of 16 partitions (one per Q7 core). Writebacks batch 28 s-columns
(4 chunks) into one contiguous-per-partition HWDGE DMA, overlapping later
gathers. Every instruction carries at most one semaphore wait (this walrus
build rejects more).
"""

import numpy as np

N = 200_000
C = 512
D = 64
NCORES = 8
NS = N // NCORES
P = 128
S = 196
SP = P * S
GW = 7  # s-columns per dma_gather chunk (896 descs < 1024 ring)
NCH = S // GW  # 28 gather chunks
CHUNK = P * GW  # 896 indices per chunk
WB = 28  # s-columns per writeback (= 4 gather chunks)
NWB = S // WB

_RUN_OPTS: dict = {}
_LAST_RESULT = None
_LAST_IN_MAPS = None
_NC_CACHE = None


def _build():
    global _NC_CACHE
    if _NC_CACHE is not None:
        return _NC_CACHE
    import concourse.bass as bass
    import concourse.mybir as mybir
    from concourse import library_config
    from concourse.library_overlay import lower_extended_insts

    nc = bass.Bass()
    idx = nc.dram_tensor("idx", [P, SP // 16], mybir.dt.int16, kind="ExternalInput")
    feat = nc.dram_tensor("feature", [C, D], mybir.dt.float32, kind="ExternalInput")
    out = nc.dram_tensor("out", [SP, D], mybir.dt.float32, kind="ExternalOutput")
    out_v = out[:].rearrange("(p s) d -> p (s d)", p=P)

    with (
        nc.sbuf_tensor("xi", [P, SP // 16], mybir.dt.int16) as xi,
        nc.sbuf_tensor("g", [P, S * D], mybir.dt.float32) as g,
        nc.semaphore("s_load") as s_load,
        nc.semaphore("s_gath") as s_gath,
        nc.semaphore("s_out") as s_out,
        nc.Block() as block,
    ):
        @block.sync
        def _(sync):
            sync.dma_start(out=xi[:], in_=idx[:]).then_inc(s_load, 16)
            for w in range(NWB):
                # chunks complete in FIFO ring order; writeback w needs the
                # first 4*(w+1) chunks done
                sync.wait_ge(s_gath, 16 * (WB // GW) * (w + 1))
                sync.dma_start(
                    out=out_v[:, w * WB * D : (w + 1) * WB * D],
                    in_=g[:, w * WB * D : (w + 1) * WB * D],
                ).then_inc(s_out, 16)
            sync.wait_ge(s_out, 16 * NWB)

        @block.gpsimd
        def _(gpsimd):
            gpsimd.load_library(library_config.mlp)
            gpsimd.wait_ge(s_load, 16)
            for j in range(NCH):
                gpsimd.dma_gather(
                    g[:, j * GW * D : (j + 1) * GW * D].rearrange(
                        "p (s d) -> p s d", d=D
                    ),
                    feat[:],
                    xi[:, j * (CHUNK // 16) : (j + 1) * (CHUNK // 16)],
                    CHUNK,
                    CHUNK,
                    D,
                    single_packet=False,
                ).then_inc(s_gath, 16)

    lower_extended_insts(nc)
    _NC_CACHE = nc
    return nc


def kernel(x, feature_array):
    global _LAST_RESULT, _LAST_IN_MAPS
    from concourse.bass_utils import run_bass_kernel_spmd

    nc = _build()
    xs = np.asarray(x).reshape(NCORES, NS).astype(np.int16)
    feat = np.ascontiguousarray(np.asarray(feature_array, dtype=np.float32))
    in_maps = []
    for i in range(NCORES):
        xp = np.zeros((P, S), dtype=np.int16)
        xp.reshape(-1)[:NS] = xs[i]
        # logical gather position i = s*128 + p -> out slot [p, s] -> row
        # p*196+s, so idx_flat = xp.T.ravel(); wrap by 16 and replicate
        # across the 8 Q7-core partition groups.
        idx_flat = np.ascontiguousarray(xp.T).ravel()
        base = idx_flat.reshape(SP // 16, 16).T
        idx_sb = np.tile(base, (8, 1))
        in_maps.append({"idx": np.ascontiguousarray(idx_sb), "feature": feat})
    _LAST_IN_MAPS = in_maps
    res = run_bass_kernel_spmd(nc, in_maps, core_ids=list(range(NCORES)), **_RUN_OPTS)
    _LAST_RESULT = res
    return np.concatenate([r["out"][:NS] for r in res.results], axis=0)
